# revision 20
# baseline (speedup 1.0000x reference)
"""Grouped linear (MoE expert GEMM) on 8 NeuronCores, expert-parallel.

Problem: hidden_states [16384, 2048] f32, weight [8, 2048, 2048] f32,
tokens_per_expert [8] = 2048 each (balanced). Output [16384, 2048] f32 with
out[g*2048+t, o] = sum_i x[g*2048+t, i] * weight[g, o, i].

Sharding: expert-parallel -- core g gets expert g's weight [2048, 2048] and its
2048 routed tokens; each core runs one 2048x2048x2048 GEMM. No collectives.

Per-core kernel, mixed precision to beat the 1-col/cycle PE floor:
- k 0..1535 (12 chunks of 128) in fp16: 1 col/cycle, 216 ns per 512-wide MM.
- k 1536..2047 (2 pairs of 256) in fp8-e4m3 with perf_mode=DoubleRow:
  2 MACs/cell/cycle, ~109 ns per 512-wide MM covering 256 k.
Both sections accumulate into one PSUM group: all operands carry a shared
power-of-2 scale (x*32, w*8192 -> PSUM holds 2^18 * out), removed by a
tensor_scalar_mul(2^-18) in the PSUM->SBUF copy. Host-simulated rel err on
the real data: 1.63e-2 (gate 2e-2); fp8 quantization dominates.

The loop is ordered (tt, km, oi) so each stationary tile is reused for 4
consecutive matmuls (LDWEIGHTS amortized/hidden). DMA trigger order is the
ramp-critical path: x(tt=0) then all W tiles, then x1..x15. Output is bf16,
batched one DMA per token tile; host upcasts.
"""

import numpy as np

G = 8
TPG = 2048  # tokens per expert (= per core)
IN = 2048
OUT = 2048
P = 128
TT = TPG // P  # 16 token tiles of 128
ON = 4  # number of output-column chunks
OW = OUT // ON  # 512
K16 = 12  # fp16 contraction chunks of 128 (k 0..1535)
KP8 = 2  # fp8 DoubleRow pairs of 256 (k 1536..2047)
SX = 32.0  # power-of-2 scale on x (both sections)
SW = 8192.0  # power-of-2 scale on w (both sections)
DEQ = 1.0 / (SX * SW)  # 2^-18

_nc_cache = {}


def _build_nc():
    import concourse.bacc as bacc
    import concourse.mybir as mybir
    import concourse.tile as tile

    if "nc" in _nc_cache:
        return _nc_cache["nc"]

    f32 = mybir.dt.float32
    bf16 = mybir.dt.bfloat16
    fp16 = mybir.dt.float16
    fp8 = mybir.dt.float8e4
    DR = mybir.MatmulPerfMode.DoubleRow

    nc = bacc.Bacc(None, target_bir_lowering=False)

    # x16[p, tt, km, t] = SX * x[tt*128+t, km*128+p]          (k on partitions)
    x16 = nc.dram_tensor("x16", [P, TT, K16, P], fp16, kind="ExternalInput")
    # w16[p, km, o] = SW * w[o, km*128+p]
    w16 = nc.dram_tensor("w16", [P, K16, OUT], fp16, kind="ExternalInput")
    # x8[p, tt, kp, i, t] = q(SX * x[tt*128+t, 1536 + kp*256 + i*128 + p])
    x8 = nc.dram_tensor("x8", [P, TT, KP8, 2, P], fp8, kind="ExternalInput")
    # w8[p, kp, i, o] = q(SW * w[o, 1536 + kp*256 + i*128 + p])
    w8 = nc.dram_tensor("w8", [P, KP8, 2, OUT], fp8, kind="ExternalInput")
    # out[tt, p, o] = C[tt*128+p, o] (bf16; host upcasts)
    out = nc.dram_tensor("out", [TT, P, OUT], bf16, kind="ExternalOutput")

    with tile.TileContext(nc) as tc:
        with (
            tc.tile_pool(name="xpool", bufs=1) as xpool,
            tc.tile_pool(name="wpool", bufs=1) as wpool,
            tc.tile_pool(name="opool", bufs=2) as opool,
            tc.tile_pool(name="ppool", bufs=8, space="PSUM") as ppool,
        ):
            OH = 2 * OW  # output half-width: 1024
            x16t = [
                xpool.tile([P, K16, P], fp16, name=f"x16_{i}", tag=f"x16_{i}")
                for i in range(TT)
            ]
            x8t = [
                xpool.tile([P, KP8, 2, P], fp8, name=f"x8_{i}", tag=f"x8_{i}")
                for i in range(TT)
            ]
            # W tiles are split into output-column halves: the whole kernel
            # runs as two phases (columns 0:1024, then 1024:2048), so only
            # ~3.75 MB of W must land before phase-0 compute; the other half
            # streams in during phase 0's ~96 us.
            w16t = [
                [
                    wpool.tile([P, OH], fp16, name=f"w16_{k}_{h}", tag=f"w16_{k}_{h}")
                    for h in range(2)
                ]
                for k in range(K16)
            ]
            w8t = [
                [
                    wpool.tile([P, 2, OH], fp8, name=f"w8_{k}_{h}", tag=f"w8_{k}_{h}")
                    for h in range(2)
                ]
                for k in range(KP8)
            ]

            def dma_x(i):
                nc.sync.dma_start(out=x16t[i][:], in_=x16[:, i])
                nc.sync.dma_start(out=x8t[i][:], in_=x8[:, i])

            # W triggers ride the ScalarE DMA queue, X the SP queue: two
            # hardware queues issue + transfer concurrently, so the first
            # matmul's two dependencies (x0, w16_0) don't serialize and the
            # ramp-critical W stream doesn't compete with X descriptors.
            def dma_w16(km, h):
                nc.scalar.dma_start(
                    out=w16t[km][h][:], in_=w16[:, km, h * OH : (h + 1) * OH]
                )

            def dma_w8(kp, h):
                nc.scalar.dma_start(
                    out=w8t[kp][h][:], in_=w8[:, kp, :, h * OH : (h + 1) * OH]
                )

            # Trigger order tuned so each phase-0 unit's data lands just
            # ahead of its compute (units consume ~620 MB/s vs the ~410 GB/s
            # DMA ceiling, so the early x tiles interleave into the W stream).
            dma_x(0)
            for km in range(3):
                dma_w16(km, 0)
            dma_x(1)
            for km in range(3, K16):
                dma_w16(km, 0)
            dma_w8(0, 0)
            dma_w8(1, 0)
            dma_x(2)
            for km in range(6):
                dma_w16(km, 1)
            dma_x(3)
            dma_x(4)
            for km in range(6, K16):
                dma_w16(km, 1)
            dma_w8(0, 1)
            dma_w8(1, 1)
            for i in range(5, TT):
                dma_x(i)

            class Unit:
                """One compute unit: token tile tt, output cols
                [obase*OW, (obase+nseg)*OW), accumulation group per oi seg."""

                def __init__(self, tt, obase, nseg):
                    self.tt, self.obase, self.nseg = tt, obase, nseg
                    self.psums = [
                        ppool.tile(
                            [P, OW], f32, name=f"ps{tt}_{obase}_{oi}", tag="ps"
                        )
                        for oi in range(nseg)
                    ]
                    self.o_sb = opool.tile(
                        [P, nseg * OW], bf16, name=f"o{tt}_{obase}", tag="o"
                    )

                def mm16(self, oi_range, km):
                    for oi in oi_range:
                        o0 = (self.obase + oi) * OW
                        nc.tensor.matmul(
                            out=self.psums[oi][:],
                            lhsT=x16t[self.tt][:, km, :],
                            rhs=w16t[km][(self.obase + oi) // 2][
                                :, o0 % OH : o0 % OH + OW
                            ],
                            start=(km == 0),
                            stop=False,
                        )

                def mm8(self, oi_range, kp):
                    for oi in oi_range:
                        o0 = (self.obase + oi) * OW
                        nc.tensor.matmul(
                            out=self.psums[oi][:],
                            lhsT=x8t[self.tt][:, kp, :, :],
                            rhs=w8t[kp][(self.obase + oi) // 2][
                                :, :, o0 % OH : o0 % OH + OW
                            ],
                            start=False,
                            stop=(kp == KP8 - 1),
                            perf_mode=DR,
                        )

                def fp16_part(self):
                    for km in range(K16):
                        self.mm16(range(self.nseg), km)

                def dr_part(self):
                    for kp in range(KP8):
                        self.mm8(range(self.nseg), kp)

                def copy_out(self, oi):
                    # Dequant copies split across DVE and ScalarE (both can
                    # read PSUM, different banks) so a unit's 4 copies clear
                    # in ~2 copy-times, not 4 — the next-but-one unit's
                    # start=True matmuls wait on these.
                    dst = self.o_sb[:, oi * OW : (oi + 1) * OW]
                    if oi % 2 == 0:
                        nc.vector.tensor_scalar_mul(dst, self.psums[oi][:], DEQ)
                    else:
                        nc.scalar.activation(
                            dst,
                            self.psums[oi][:],
                            mybir.ActivationFunctionType.Copy,
                            scale=DEQ,
                        )

                def finish(self):
                    for oi in range(self.nseg):
                        self.copy_out(oi)
                    nc.scalar.dma_start(
                        out=out[
                            self.tt,
                            :,
                            self.obase * OW : (self.obase + self.nseg) * OW,
                        ],
                        in_=self.o_sb[:],
                    )

            def run_units(units):
                """Pair units so their DR sections run back-to-back: the
                fp16->DR weight-buffer transition costs ~190 ns of PE time
                (a DoubleRow LDWEIGHTS fills both weight slots so it cannot
                prefetch behind fp16 matmuls); pairing halves that count."""
                for i in range(0, len(units) - 1, 2):
                    a, b = units[i], units[i + 1]
                    a.fp16_part()
                    b.fp16_part()
                    a.dr_part()
                    b.dr_part()
                    a.finish()
                    b.finish()
                if len(units) % 2:
                    u = units[-1]
                    u.fp16_part()
                    u.dr_part()
                    u.finish()

            def tail_unit(tt):
                # Tail shape: per-oi groups so copies/DMAs overlap the
                # remaining matmuls; the very last copy splits across both
                # engines (half-width each) to shorten the critical tail.
                u = Unit(tt, 0, ON)
                for oi in range(ON):
                    for km in range(K16):
                        u.mm16([oi], km)
                    for kp in range(KP8):
                        u.mm8([oi], kp)
                    if oi == ON - 1:
                        hw = OW // 2
                        nc.vector.tensor_scalar_mul(
                            u.o_sb[:, oi * OW : oi * OW + hw],
                            u.psums[oi][:, :hw],
                            DEQ,
                        )
                        nc.scalar.activation(
                            u.o_sb[:, oi * OW + hw : (oi + 1) * OW],
                            u.psums[oi][:, hw:],
                            mybir.ActivationFunctionType.Copy,
                            scale=DEQ,
                        )
                    else:
                        u.copy_out(oi)
                    nc.scalar.dma_start(
                        out=out[tt, :, oi * OW : (oi + 1) * OW],
                        in_=u.o_sb[:, oi * OW : (oi + 1) * OW],
                    )

            # Ramp phase: half-width units over tt0..3 while W streams in,
            # paired as well (also defers the w8 dependency deeper into the
            # ramp); then paired full-width units.
            RAMP_TT = 4
            run_units([Unit(tt, 0, 2) for tt in range(RAMP_TT)])
            run_units([Unit(tt, 2, 2) for tt in range(RAMP_TT)])
            run_units([Unit(tt, 0, ON) for tt in range(RAMP_TT, TT - 1)])
            tail_unit(TT - 1)

    nc.compile()
    _nc_cache["nc"] = nc
    return nc


def _shard_inputs(hidden_states, weight):
    """Host-side quantize + reshuffle into the kernel's DRAM layouts."""
    import ml_dtypes

    fp8 = ml_dtypes.float8_e4m3  # IEEE e4m3, max 240 == TRN FP8_EXP4
    x = np.asarray(hidden_states, dtype=np.float32)
    w = np.asarray(weight, dtype=np.float32)
    k16 = K16 * P  # 1536
    in_maps = []
    for g in range(G):
        xg = x[g * TPG : (g + 1) * TPG]  # [2048, 2048]
        wg = w[g]  # [out, in]
        # fp16 section, k < 1536: [tt, t, km, p] -> [p, tt, km, t]
        x16 = np.ascontiguousarray(
            (xg[:, :k16] * SX)
            .reshape(TT, P, K16, P)
            .transpose(3, 0, 2, 1)
            .astype(np.float16)
        )
        w16 = np.ascontiguousarray(
            (wg[:, :k16] * SW)
            .reshape(OUT, K16, P)
            .transpose(2, 1, 0)
            .astype(np.float16)
        )
        # fp8 section, k >= 1536: [tt, t, kp, i, p] -> [p, tt, kp, i, t]
        x8 = np.ascontiguousarray(
            np.clip(xg[:, k16:] * SX, -240.0, 240.0)
            .reshape(TT, P, KP8, 2, P)
            .transpose(4, 0, 2, 3, 1)
            .astype(fp8)
        )
        w8 = np.ascontiguousarray(
            np.clip(wg[:, k16:] * SW, -240.0, 240.0)
            .reshape(OUT, KP8, 2, P)
            .transpose(3, 1, 2, 0)
            .astype(fp8)
        )
        in_maps.append({"x16": x16, "w16": w16, "x8": x8, "w8": w8})
    return in_maps


def _run(hidden_states, weight, trace=False, tmpdir=None):
    from concourse.bass_utils import run_bass_kernel_spmd

    nc = _build_nc()
    in_maps = _shard_inputs(hidden_states, weight)
    res = run_bass_kernel_spmd(
        nc, in_maps, core_ids=list(range(G)), trace=trace, tmpdir=tmpdir
    )
    outs = [
        np.asarray(res.results[g]["out"]).astype(np.float32).reshape(TPG, OUT)
        for g in range(G)
    ]
    full = np.concatenate(outs, axis=0)
    return full, res


def kernel(hidden_states, weight, tokens_per_expert=None, **_ignored):
    out, _ = _run(hidden_states, weight, trace=False)
    return out


# revision 22
# speedup vs baseline: 1.0130x; 1.0130x over previous
"""Grouped linear (MoE expert GEMM) on 8 NeuronCores, expert-parallel.

Problem: hidden_states [16384, 2048] f32, weight [8, 2048, 2048] f32,
tokens_per_expert [8] = 2048 each (balanced). Output [16384, 2048] f32 with
out[g*2048+t, o] = sum_i x[g*2048+t, i] * weight[g, o, i].

Sharding: expert-parallel -- core g gets expert g's weight [2048, 2048] and its
2048 routed tokens; each core runs one 2048x2048x2048 GEMM. No collectives.

Per-core kernel, mixed precision to beat the 1-col/cycle PE floor:
- k 0..1535 (12 chunks of 128) in fp16: 1 col/cycle, 216 ns per 512-wide MM.
- k 1536..2047 (2 pairs of 256) in fp8-e4m3 with perf_mode=DoubleRow:
  2 MACs/cell/cycle, ~109 ns per 512-wide MM covering 256 k.
Both sections accumulate into one PSUM group: all operands carry a shared
power-of-2 scale (x*32, w*8192 -> PSUM holds 2^18 * out), removed by a
tensor_scalar_mul(2^-18) in the PSUM->SBUF copy. Host-simulated rel err on
the real data: 1.63e-2 (gate 2e-2); fp8 quantization dominates.

The loop is ordered (tt, km, oi) so each stationary tile is reused for 4
consecutive matmuls (LDWEIGHTS amortized/hidden). DMA trigger order is the
ramp-critical path: x(tt=0) then all W tiles, then x1..x15. Output is bf16,
batched one DMA per token tile; host upcasts.
"""

import numpy as np

G = 8
TPG = 2048  # tokens per expert (= per core)
IN = 2048
OUT = 2048
P = 128
TT = TPG // P  # 16 token tiles of 128
ON = 4  # number of output-column chunks
OW = OUT // ON  # 512
K16 = 12  # fp16 contraction chunks of 128 (k 0..1535)
KP8 = 2  # fp8 DoubleRow pairs of 256 (k 1536..2047)
SX = 32.0  # power-of-2 scale on x (both sections)
SW = 8192.0  # power-of-2 scale on w (both sections)
DEQ = 1.0 / (SX * SW)  # 2^-18

_nc_cache = {}


def _build_nc():
    import concourse.bacc as bacc
    import concourse.mybir as mybir
    import concourse.tile as tile

    if "nc" in _nc_cache:
        return _nc_cache["nc"]

    f32 = mybir.dt.float32
    bf16 = mybir.dt.bfloat16
    fp16 = mybir.dt.float16
    fp8 = mybir.dt.float8e4
    DR = mybir.MatmulPerfMode.DoubleRow

    nc = bacc.Bacc(None, target_bir_lowering=False)

    # x16[p, tt, km, t] = SX * x[tt*128+t, km*128+p]          (k on partitions)
    x16 = nc.dram_tensor("x16", [P, TT, K16, P], fp16, kind="ExternalInput")
    # w16[p, km, o] = SW * w[o, km*128+p]
    w16 = nc.dram_tensor("w16", [P, K16, OUT], fp16, kind="ExternalInput")
    # x8[p, tt, kp, i, t] = q(SX * x[tt*128+t, 1536 + kp*256 + i*128 + p])
    x8 = nc.dram_tensor("x8", [P, TT, KP8, 2, P], fp8, kind="ExternalInput")
    # w8[p, kp, i, o] = q(SW * w[o, 1536 + kp*256 + i*128 + p])
    w8 = nc.dram_tensor("w8", [P, KP8, 2, OUT], fp8, kind="ExternalInput")
    # out[tt, p, o] = C[tt*128+p, o] (bf16; host upcasts)
    out = nc.dram_tensor("out", [TT, P, OUT], bf16, kind="ExternalOutput")

    with tile.TileContext(nc) as tc:
        with (
            tc.tile_pool(name="xpool", bufs=1) as xpool,
            tc.tile_pool(name="wpool", bufs=1) as wpool,
            tc.tile_pool(name="opool", bufs=2) as opool,
            tc.tile_pool(name="ppool", bufs=8, space="PSUM") as ppool,
        ):
            OH = 2 * OW  # output half-width: 1024
            x16t = [
                xpool.tile([P, K16, P], fp16, name=f"x16_{i}", tag=f"x16_{i}")
                for i in range(TT)
            ]
            x8t = [
                xpool.tile([P, KP8, 2, P], fp8, name=f"x8_{i}", tag=f"x8_{i}")
                for i in range(TT)
            ]
            # W tiles are split into output-column halves: the whole kernel
            # runs as two phases (columns 0:1024, then 1024:2048), so only
            # ~3.75 MB of W must land before phase-0 compute; the other half
            # streams in during phase 0's ~96 us.
            w16t = [
                [
                    wpool.tile([P, OH], fp16, name=f"w16_{k}_{h}", tag=f"w16_{k}_{h}")
                    for h in range(2)
                ]
                for k in range(K16)
            ]
            w8t = [
                [
                    wpool.tile([P, 2, OH], fp8, name=f"w8_{k}_{h}", tag=f"w8_{k}_{h}")
                    for h in range(2)
                ]
                for k in range(KP8)
            ]

            def dma_x(i):
                eng = nc.scalar if i == 0 else nc.sync
                eng.dma_start(out=x16t[i][:], in_=x16[:, i])
                eng.dma_start(out=x8t[i][:], in_=x8[:, i])

            # All bulk input DMAs stay on one queue (SP) in exact consumption
            # order — a second queue racing ahead starves the ramp-critical
            # W stream. Only tt0's x tiles ride the ScalarE queue, so the
            # first matmul's two dependencies (x0, w16_0) transfer in
            # parallel instead of back-to-back.
            def dma_w16(km, h):
                nc.sync.dma_start(
                    out=w16t[km][h][:], in_=w16[:, km, h * OH : (h + 1) * OH]
                )

            def dma_w8(kp, h):
                nc.sync.dma_start(
                    out=w8t[kp][h][:], in_=w8[:, kp, :, h * OH : (h + 1) * OH]
                )

            # Trigger order tuned so each phase-0 unit's data lands just
            # ahead of its compute (units consume ~620 MB/s vs the ~410 GB/s
            # DMA ceiling, so the early x tiles interleave into the W stream).
            dma_x(0)
            for km in range(3):
                dma_w16(km, 0)
            dma_x(1)
            for km in range(3, K16):
                dma_w16(km, 0)
            dma_w8(0, 0)
            dma_w8(1, 0)
            dma_x(2)
            for km in range(6):
                dma_w16(km, 1)
            dma_x(3)
            dma_x(4)
            for km in range(6, K16):
                dma_w16(km, 1)
            dma_w8(0, 1)
            dma_w8(1, 1)
            for i in range(5, TT):
                dma_x(i)

            class Unit:
                """One compute unit: token tile tt, output cols
                [obase*OW, (obase+nseg)*OW), accumulation group per oi seg."""

                def __init__(self, tt, obase, nseg):
                    self.tt, self.obase, self.nseg = tt, obase, nseg
                    self.psums = [
                        ppool.tile(
                            [P, OW], f32, name=f"ps{tt}_{obase}_{oi}", tag="ps"
                        )
                        for oi in range(nseg)
                    ]
                    self.o_sb = opool.tile(
                        [P, nseg * OW], bf16, name=f"o{tt}_{obase}", tag="o"
                    )

                def mm16(self, oi_range, km):
                    for oi in oi_range:
                        o0 = (self.obase + oi) * OW
                        nc.tensor.matmul(
                            out=self.psums[oi][:],
                            lhsT=x16t[self.tt][:, km, :],
                            rhs=w16t[km][(self.obase + oi) // 2][
                                :, o0 % OH : o0 % OH + OW
                            ],
                            start=(km == 0),
                            stop=False,
                        )

                def mm8(self, oi_range, kp):
                    for oi in oi_range:
                        o0 = (self.obase + oi) * OW
                        nc.tensor.matmul(
                            out=self.psums[oi][:],
                            lhsT=x8t[self.tt][:, kp, :, :],
                            rhs=w8t[kp][(self.obase + oi) // 2][
                                :, :, o0 % OH : o0 % OH + OW
                            ],
                            start=False,
                            stop=(kp == KP8 - 1),
                            perf_mode=DR,
                        )

                def fp16_part(self):
                    for km in range(K16):
                        self.mm16(range(self.nseg), km)

                def dr_part(self):
                    for kp in range(KP8):
                        self.mm8(range(self.nseg), kp)

                def copy_out(self, oi):
                    # Dequant copies split across DVE and ScalarE (both can
                    # read PSUM, different banks) so a unit's 4 copies clear
                    # in ~2 copy-times, not 4 — the next-but-one unit's
                    # start=True matmuls wait on these.
                    dst = self.o_sb[:, oi * OW : (oi + 1) * OW]
                    if oi % 2 == 0:
                        nc.vector.tensor_scalar_mul(dst, self.psums[oi][:], DEQ)
                    else:
                        nc.scalar.activation(
                            dst,
                            self.psums[oi][:],
                            mybir.ActivationFunctionType.Copy,
                            scale=DEQ,
                        )

                def finish(self):
                    for oi in range(self.nseg):
                        self.copy_out(oi)
                    nc.scalar.dma_start(
                        out=out[
                            self.tt,
                            :,
                            self.obase * OW : (self.obase + self.nseg) * OW,
                        ],
                        in_=self.o_sb[:],
                    )

            def run_units(units):
                """Pair units so their DR sections run back-to-back: the
                fp16->DR weight-buffer transition costs ~190 ns of PE time
                (a DoubleRow LDWEIGHTS fills both weight slots so it cannot
                prefetch behind fp16 matmuls); pairing halves that count."""
                for i in range(0, len(units) - 1, 2):
                    a, b = units[i], units[i + 1]
                    a.fp16_part()
                    b.fp16_part()
                    a.dr_part()
                    b.dr_part()
                    a.finish()
                    b.finish()
                if len(units) % 2:
                    u = units[-1]
                    u.fp16_part()
                    u.dr_part()
                    u.finish()

            def tail_unit(tt):
                # Tail shape: per-oi groups so copies/DMAs overlap the
                # remaining matmuls; the very last copy splits across both
                # engines (half-width each) to shorten the critical tail.
                u = Unit(tt, 0, ON)
                for oi in range(ON):
                    for km in range(K16):
                        u.mm16([oi], km)
                    for kp in range(KP8):
                        u.mm8([oi], kp)
                    if oi == ON - 1:
                        hw = OW // 2
                        nc.vector.tensor_scalar_mul(
                            u.o_sb[:, oi * OW : oi * OW + hw],
                            u.psums[oi][:, :hw],
                            DEQ,
                        )
                        nc.scalar.activation(
                            u.o_sb[:, oi * OW + hw : (oi + 1) * OW],
                            u.psums[oi][:, hw:],
                            mybir.ActivationFunctionType.Copy,
                            scale=DEQ,
                        )
                    else:
                        u.copy_out(oi)
                    nc.scalar.dma_start(
                        out=out[tt, :, oi * OW : (oi + 1) * OW],
                        in_=u.o_sb[:, oi * OW : (oi + 1) * OW],
                    )

            # Ramp phase: half-width units over tt0..3 while W streams in,
            # paired as well (also defers the w8 dependency deeper into the
            # ramp); then paired full-width units.
            RAMP_TT = 4
            run_units([Unit(tt, 0, 2) for tt in range(RAMP_TT)])
            run_units([Unit(tt, 2, 2) for tt in range(RAMP_TT)])
            run_units([Unit(tt, 0, ON) for tt in range(RAMP_TT, TT - 1)])
            tail_unit(TT - 1)

    nc.compile()
    _nc_cache["nc"] = nc
    return nc


def _shard_inputs(hidden_states, weight):
    """Host-side quantize + reshuffle into the kernel's DRAM layouts."""
    import ml_dtypes

    fp8 = ml_dtypes.float8_e4m3  # IEEE e4m3, max 240 == TRN FP8_EXP4
    x = np.asarray(hidden_states, dtype=np.float32)
    w = np.asarray(weight, dtype=np.float32)
    k16 = K16 * P  # 1536
    in_maps = []
    for g in range(G):
        xg = x[g * TPG : (g + 1) * TPG]  # [2048, 2048]
        wg = w[g]  # [out, in]
        # fp16 section, k < 1536: [tt, t, km, p] -> [p, tt, km, t]
        x16 = np.ascontiguousarray(
            (xg[:, :k16] * SX)
            .reshape(TT, P, K16, P)
            .transpose(3, 0, 2, 1)
            .astype(np.float16)
        )
        w16 = np.ascontiguousarray(
            (wg[:, :k16] * SW)
            .reshape(OUT, K16, P)
            .transpose(2, 1, 0)
            .astype(np.float16)
        )
        # fp8 section, k >= 1536: [tt, t, kp, i, p] -> [p, tt, kp, i, t]
        x8 = np.ascontiguousarray(
            np.clip(xg[:, k16:] * SX, -240.0, 240.0)
            .reshape(TT, P, KP8, 2, P)
            .transpose(4, 0, 2, 3, 1)
            .astype(fp8)
        )
        w8 = np.ascontiguousarray(
            np.clip(wg[:, k16:] * SW, -240.0, 240.0)
            .reshape(OUT, KP8, 2, P)
            .transpose(3, 1, 2, 0)
            .astype(fp8)
        )
        in_maps.append({"x16": x16, "w16": w16, "x8": x8, "w8": w8})
    return in_maps


def _run(hidden_states, weight, trace=False, tmpdir=None):
    from concourse.bass_utils import run_bass_kernel_spmd

    nc = _build_nc()
    in_maps = _shard_inputs(hidden_states, weight)
    res = run_bass_kernel_spmd(
        nc, in_maps, core_ids=list(range(G)), trace=trace, tmpdir=tmpdir
    )
    outs = [
        np.asarray(res.results[g]["out"]).astype(np.float32).reshape(TPG, OUT)
        for g in range(G)
    ]
    full = np.concatenate(outs, axis=0)
    return full, res


def kernel(hidden_states, weight, tokens_per_expert=None, **_ignored):
    out, _ = _run(hidden_states, weight, trace=False)
    return out


# revision 24
# speedup vs baseline: 1.0145x; 1.0015x over previous
"""Grouped linear (MoE expert GEMM) on 8 NeuronCores, expert-parallel.

Problem: hidden_states [16384, 2048] f32, weight [8, 2048, 2048] f32,
tokens_per_expert [8] = 2048 each (balanced). Output [16384, 2048] f32 with
out[g*2048+t, o] = sum_i x[g*2048+t, i] * weight[g, o, i].

Sharding: expert-parallel -- core g gets expert g's weight [2048, 2048] and its
2048 routed tokens; each core runs one 2048x2048x2048 GEMM. No collectives.

Per-core kernel, mixed precision to beat the 1-col/cycle PE floor:
- k 0..1535 (12 chunks of 128) in fp16: 1 col/cycle, 216 ns per 512-wide MM.
- k 1536..2047 (2 pairs of 256) in fp8-e4m3 with perf_mode=DoubleRow:
  2 MACs/cell/cycle, ~109 ns per 512-wide MM covering 256 k.
Both sections accumulate into one PSUM group: all operands carry a shared
power-of-2 scale (x*32, w*8192 -> PSUM holds 2^18 * out), removed by a
tensor_scalar_mul(2^-18) in the PSUM->SBUF copy. Host-simulated rel err on
the real data: 1.63e-2 (gate 2e-2); fp8 quantization dominates.

The loop is ordered (tt, km, oi) so each stationary tile is reused for 4
consecutive matmuls (LDWEIGHTS amortized/hidden). DMA trigger order is the
ramp-critical path: x(tt=0) then all W tiles, then x1..x15. Output is bf16,
batched one DMA per token tile; host upcasts.
"""

import numpy as np

G = 8
TPG = 2048  # tokens per expert (= per core)
IN = 2048
OUT = 2048
P = 128
TT = TPG // P  # 16 token tiles of 128
ON = 4  # number of output-column chunks
OW = OUT // ON  # 512
K16 = 12  # fp16 contraction chunks of 128 (k 0..1535)
KP8 = 2  # fp8 DoubleRow pairs of 256 (k 1536..2047)
SX = 32.0  # power-of-2 scale on x (both sections)
SW = 8192.0  # power-of-2 scale on w (both sections)
DEQ = 1.0 / (SX * SW)  # 2^-18

_nc_cache = {}


def _build_nc():
    import concourse.bacc as bacc
    import concourse.mybir as mybir
    import concourse.tile as tile

    if "nc" in _nc_cache:
        return _nc_cache["nc"]

    f32 = mybir.dt.float32
    bf16 = mybir.dt.bfloat16
    fp16 = mybir.dt.float16
    fp8 = mybir.dt.float8e4
    DR = mybir.MatmulPerfMode.DoubleRow

    nc = bacc.Bacc(None, target_bir_lowering=False)

    # x16[p, tt, km, t] = SX * x[tt*128+t, km*128+p]          (k on partitions)
    x16 = nc.dram_tensor("x16", [P, TT, K16, P], fp16, kind="ExternalInput")
    # w16[p, km, o] = SW * w[o, km*128+p]
    w16 = nc.dram_tensor("w16", [P, K16, OUT], fp16, kind="ExternalInput")
    # x8[p, tt, kp, i, t] = q(SX * x[tt*128+t, 1536 + kp*256 + i*128 + p])
    x8 = nc.dram_tensor("x8", [P, TT, KP8, 2, P], fp8, kind="ExternalInput")
    # w8[p, kp, i, o] = q(SW * w[o, 1536 + kp*256 + i*128 + p])
    w8 = nc.dram_tensor("w8", [P, KP8, 2, OUT], fp8, kind="ExternalInput")
    # out[tt, p, o] = C[tt*128+p, o] (bf16; host upcasts)
    out = nc.dram_tensor("out", [TT, P, OUT], bf16, kind="ExternalOutput")

    with tile.TileContext(nc) as tc:
        with (
            tc.tile_pool(name="xpool", bufs=1) as xpool,
            tc.tile_pool(name="wpool", bufs=1) as wpool,
            tc.tile_pool(name="opool", bufs=2) as opool,
            tc.tile_pool(name="ppool", bufs=8, space="PSUM") as ppool,
        ):
            OH = 2 * OW  # output half-width: 1024
            x16t = [
                xpool.tile([P, K16, P], fp16, name=f"x16_{i}", tag=f"x16_{i}")
                for i in range(TT)
            ]
            x8t = [
                xpool.tile([P, KP8, 2, P], fp8, name=f"x8_{i}", tag=f"x8_{i}")
                for i in range(TT)
            ]
            # W tiles are split into output-column halves: the whole kernel
            # runs as two phases (columns 0:1024, then 1024:2048), so only
            # ~3.75 MB of W must land before phase-0 compute; the other half
            # streams in during phase 0's ~96 us.
            w16t = [
                [
                    wpool.tile([P, OH], fp16, name=f"w16_{k}_{h}", tag=f"w16_{k}_{h}")
                    for h in range(2)
                ]
                for k in range(K16)
            ]
            w8t = [
                [
                    wpool.tile([P, 2, OH], fp8, name=f"w8_{k}_{h}", tag=f"w8_{k}_{h}")
                    for h in range(2)
                ]
                for k in range(KP8)
            ]

            def dma_x(i):
                nc.sync.dma_start(out=x16t[i][:], in_=x16[:, i])
                nc.sync.dma_start(out=x8t[i][:], in_=x8[:, i])

            # All bulk input DMAs stay on one queue (SP) in exact consumption
            # order — a second queue racing ahead starves the ramp-critical
            # W stream. Only tt0's x tiles ride the ScalarE queue, so the
            # first matmul's two dependencies (x0, w16_0) transfer in
            # parallel instead of back-to-back.
            def dma_w16(km, h):
                nc.sync.dma_start(
                    out=w16t[km][h][:], in_=w16[:, km, h * OH : (h + 1) * OH]
                )

            def dma_w8(kp, h):
                nc.sync.dma_start(
                    out=w8t[kp][h][:], in_=w8[:, kp, :, h * OH : (h + 1) * OH]
                )

            # Trigger order tuned so each phase-0 unit's data lands just
            # ahead of its compute (units consume ~620 MB/s vs the ~410 GB/s
            # DMA ceiling, so the early x tiles interleave into the W stream).
            # x16_0 (the first matmul's 0.875 MB long pole) rides the ScalarE
            # queue alone, in parallel with the W stream on SP; x8_0 is not
            # needed until ~10 us after MM0, so it joins the ordered stream.
            nc.scalar.dma_start(out=x16t[0][:], in_=x16[:, 0])
            for km in range(3):
                dma_w16(km, 0)
            nc.sync.dma_start(out=x8t[0][:], in_=x8[:, 0])
            dma_x(1)
            for km in range(3, K16):
                dma_w16(km, 0)
            dma_w8(0, 0)
            dma_w8(1, 0)
            dma_x(2)
            for km in range(6):
                dma_w16(km, 1)
            dma_x(3)
            dma_x(4)
            for km in range(6, K16):
                dma_w16(km, 1)
            dma_w8(0, 1)
            dma_w8(1, 1)
            for i in range(5, TT):
                dma_x(i)

            class Unit:
                """One compute unit: token tile tt, output cols
                [obase*OW, (obase+nseg)*OW), accumulation group per oi seg."""

                def __init__(self, tt, obase, nseg):
                    self.tt, self.obase, self.nseg = tt, obase, nseg
                    self.psums = [
                        ppool.tile(
                            [P, OW], f32, name=f"ps{tt}_{obase}_{oi}", tag="ps"
                        )
                        for oi in range(nseg)
                    ]
                    self.o_sb = opool.tile(
                        [P, nseg * OW], bf16, name=f"o{tt}_{obase}", tag="o"
                    )

                def mm16(self, oi_range, km):
                    for oi in oi_range:
                        o0 = (self.obase + oi) * OW
                        nc.tensor.matmul(
                            out=self.psums[oi][:],
                            lhsT=x16t[self.tt][:, km, :],
                            rhs=w16t[km][(self.obase + oi) // 2][
                                :, o0 % OH : o0 % OH + OW
                            ],
                            start=(km == 0),
                            stop=False,
                        )

                def mm8(self, oi_range, kp):
                    for oi in oi_range:
                        o0 = (self.obase + oi) * OW
                        nc.tensor.matmul(
                            out=self.psums[oi][:],
                            lhsT=x8t[self.tt][:, kp, :, :],
                            rhs=w8t[kp][(self.obase + oi) // 2][
                                :, :, o0 % OH : o0 % OH + OW
                            ],
                            start=False,
                            stop=(kp == KP8 - 1),
                            perf_mode=DR,
                        )

                def fp16_part(self):
                    for km in range(K16):
                        self.mm16(range(self.nseg), km)

                def dr_part(self):
                    for kp in range(KP8):
                        self.mm8(range(self.nseg), kp)

                def copy_out(self, oi):
                    # Dequant copies split across DVE and ScalarE (both can
                    # read PSUM, different banks) so a unit's 4 copies clear
                    # in ~2 copy-times, not 4 — the next-but-one unit's
                    # start=True matmuls wait on these.
                    dst = self.o_sb[:, oi * OW : (oi + 1) * OW]
                    if oi % 2 == 0:
                        nc.vector.tensor_scalar_mul(dst, self.psums[oi][:], DEQ)
                    else:
                        nc.scalar.activation(
                            dst,
                            self.psums[oi][:],
                            mybir.ActivationFunctionType.Copy,
                            scale=DEQ,
                        )

                def finish(self):
                    for oi in range(self.nseg):
                        self.copy_out(oi)
                    nc.scalar.dma_start(
                        out=out[
                            self.tt,
                            :,
                            self.obase * OW : (self.obase + self.nseg) * OW,
                        ],
                        in_=self.o_sb[:],
                    )

            def run_units(units):
                """Pair units so their DR sections run back-to-back: the
                fp16->DR weight-buffer transition costs ~190 ns of PE time
                (a DoubleRow LDWEIGHTS fills both weight slots so it cannot
                prefetch behind fp16 matmuls); pairing halves that count."""
                for i in range(0, len(units) - 1, 2):
                    a, b = units[i], units[i + 1]
                    a.fp16_part()
                    b.fp16_part()
                    a.dr_part()
                    b.dr_part()
                    a.finish()
                    b.finish()
                if len(units) % 2:
                    u = units[-1]
                    u.fp16_part()
                    u.dr_part()
                    u.finish()

            def tail_unit(tt):
                # Tail shape: per-oi groups so copies/DMAs overlap the
                # remaining matmuls; the very last copy splits across both
                # engines (half-width each) to shorten the critical tail.
                u = Unit(tt, 0, ON)
                for oi in range(ON):
                    for km in range(K16):
                        u.mm16([oi], km)
                    for kp in range(KP8):
                        u.mm8([oi], kp)
                    if oi == ON - 1:
                        hw = OW // 2
                        nc.vector.tensor_scalar_mul(
                            u.o_sb[:, oi * OW : oi * OW + hw],
                            u.psums[oi][:, :hw],
                            DEQ,
                        )
                        nc.scalar.activation(
                            u.o_sb[:, oi * OW + hw : (oi + 1) * OW],
                            u.psums[oi][:, hw:],
                            mybir.ActivationFunctionType.Copy,
                            scale=DEQ,
                        )
                    else:
                        u.copy_out(oi)
                    nc.scalar.dma_start(
                        out=out[tt, :, oi * OW : (oi + 1) * OW],
                        in_=u.o_sb[:, oi * OW : (oi + 1) * OW],
                    )

            # Ramp phase: half-width units over tt0..3 while W streams in,
            # paired as well (also defers the w8 dependency deeper into the
            # ramp); then paired full-width units.
            RAMP_TT = 4
            run_units([Unit(tt, 0, 2) for tt in range(RAMP_TT)])
            run_units([Unit(tt, 2, 2) for tt in range(RAMP_TT)])
            run_units([Unit(tt, 0, ON) for tt in range(RAMP_TT, TT - 1)])
            tail_unit(TT - 1)

    nc.compile()
    _nc_cache["nc"] = nc
    return nc


def _shard_inputs(hidden_states, weight):
    """Host-side quantize + reshuffle into the kernel's DRAM layouts."""
    import ml_dtypes

    fp8 = ml_dtypes.float8_e4m3  # IEEE e4m3, max 240 == TRN FP8_EXP4
    x = np.asarray(hidden_states, dtype=np.float32)
    w = np.asarray(weight, dtype=np.float32)
    k16 = K16 * P  # 1536
    in_maps = []
    for g in range(G):
        xg = x[g * TPG : (g + 1) * TPG]  # [2048, 2048]
        wg = w[g]  # [out, in]
        # fp16 section, k < 1536: [tt, t, km, p] -> [p, tt, km, t]
        x16 = np.ascontiguousarray(
            (xg[:, :k16] * SX)
            .reshape(TT, P, K16, P)
            .transpose(3, 0, 2, 1)
            .astype(np.float16)
        )
        w16 = np.ascontiguousarray(
            (wg[:, :k16] * SW)
            .reshape(OUT, K16, P)
            .transpose(2, 1, 0)
            .astype(np.float16)
        )
        # fp8 section, k >= 1536: [tt, t, kp, i, p] -> [p, tt, kp, i, t]
        x8 = np.ascontiguousarray(
            np.clip(xg[:, k16:] * SX, -240.0, 240.0)
            .reshape(TT, P, KP8, 2, P)
            .transpose(4, 0, 2, 3, 1)
            .astype(fp8)
        )
        w8 = np.ascontiguousarray(
            np.clip(wg[:, k16:] * SW, -240.0, 240.0)
            .reshape(OUT, KP8, 2, P)
            .transpose(3, 1, 2, 0)
            .astype(fp8)
        )
        in_maps.append({"x16": x16, "w16": w16, "x8": x8, "w8": w8})
    return in_maps


def _run(hidden_states, weight, trace=False, tmpdir=None):
    from concourse.bass_utils import run_bass_kernel_spmd

    nc = _build_nc()
    in_maps = _shard_inputs(hidden_states, weight)
    res = run_bass_kernel_spmd(
        nc, in_maps, core_ids=list(range(G)), trace=trace, tmpdir=tmpdir
    )
    outs = [
        np.asarray(res.results[g]["out"]).astype(np.float32).reshape(TPG, OUT)
        for g in range(G)
    ]
    full = np.concatenate(outs, axis=0)
    return full, res


def kernel(hidden_states, weight, tokens_per_expert=None, **_ignored):
    out, _ = _run(hidden_states, weight, trace=False)
    return out


# revision 25
# speedup vs baseline: 1.0157x; 1.0012x over previous
"""Grouped linear (MoE expert GEMM) on 8 NeuronCores, expert-parallel.

Problem: hidden_states [16384, 2048] f32, weight [8, 2048, 2048] f32,
tokens_per_expert [8] = 2048 each (balanced). Output [16384, 2048] f32 with
out[g*2048+t, o] = sum_i x[g*2048+t, i] * weight[g, o, i].

Sharding: expert-parallel -- core g gets expert g's weight [2048, 2048] and its
2048 routed tokens; each core runs one 2048x2048x2048 GEMM. No collectives.

Per-core kernel, mixed precision to beat the 1-col/cycle PE floor:
- k 0..1535 (12 chunks of 128) in fp16: 1 col/cycle, 216 ns per 512-wide MM.
- k 1536..2047 (2 pairs of 256) in fp8-e4m3 with perf_mode=DoubleRow:
  2 MACs/cell/cycle, ~109 ns per 512-wide MM covering 256 k.
Both sections accumulate into one PSUM group: all operands carry a shared
power-of-2 scale (x*32, w*8192 -> PSUM holds 2^18 * out), removed by a
tensor_scalar_mul(2^-18) in the PSUM->SBUF copy. Host-simulated rel err on
the real data: 1.63e-2 (gate 2e-2); fp8 quantization dominates.

The loop is ordered (tt, km, oi) so each stationary tile is reused for 4
consecutive matmuls (LDWEIGHTS amortized/hidden). DMA trigger order is the
ramp-critical path: x(tt=0) then all W tiles, then x1..x15. Output is bf16,
batched one DMA per token tile; host upcasts.
"""

import numpy as np

G = 8
TPG = 2048  # tokens per expert (= per core)
IN = 2048
OUT = 2048
P = 128
TT = TPG // P  # 16 token tiles of 128
ON = 4  # number of output-column chunks
OW = OUT // ON  # 512
K16 = 12  # fp16 contraction chunks of 128 (k 0..1535)
KP8 = 2  # fp8 DoubleRow pairs of 256 (k 1536..2047)
SX = 32.0  # power-of-2 scale on x (both sections)
SW = 8192.0  # power-of-2 scale on w (both sections)
DEQ = 1.0 / (SX * SW)  # 2^-18

_nc_cache = {}


def _build_nc():
    import concourse.bacc as bacc
    import concourse.mybir as mybir
    import concourse.tile as tile

    if "nc" in _nc_cache:
        return _nc_cache["nc"]

    f32 = mybir.dt.float32
    bf16 = mybir.dt.bfloat16
    fp16 = mybir.dt.float16
    fp8 = mybir.dt.float8e4
    DR = mybir.MatmulPerfMode.DoubleRow

    nc = bacc.Bacc(None, target_bir_lowering=False)

    # x16[p, tt, km, t] = SX * x[tt*128+t, km*128+p]          (k on partitions)
    x16 = nc.dram_tensor("x16", [P, TT, K16, P], fp16, kind="ExternalInput")
    # w16[p, km, o] = SW * w[o, km*128+p]
    w16 = nc.dram_tensor("w16", [P, K16, OUT], fp16, kind="ExternalInput")
    # x8[p, tt, kp, i, t] = q(SX * x[tt*128+t, 1536 + kp*256 + i*128 + p])
    x8 = nc.dram_tensor("x8", [P, TT, KP8, 2, P], fp8, kind="ExternalInput")
    # w8[p, kp, i, o] = q(SW * w[o, 1536 + kp*256 + i*128 + p])
    w8 = nc.dram_tensor("w8", [P, KP8, 2, OUT], fp8, kind="ExternalInput")
    # out[tt, p, o] = C[tt*128+p, o] (bf16; host upcasts)
    out = nc.dram_tensor("out", [TT, P, OUT], bf16, kind="ExternalOutput")

    with tile.TileContext(nc) as tc:
        with (
            tc.tile_pool(name="xpool", bufs=1) as xpool,
            tc.tile_pool(name="wpool", bufs=1) as wpool,
            tc.tile_pool(name="opool", bufs=2) as opool,
            tc.tile_pool(name="ppool", bufs=8, space="PSUM") as ppool,
        ):
            OH = 2 * OW  # output half-width: 1024
            x16t = [
                xpool.tile([P, K16, P], fp16, name=f"x16_{i}", tag=f"x16_{i}")
                for i in range(TT)
            ]
            x8t = [
                xpool.tile([P, KP8, 2, P], fp8, name=f"x8_{i}", tag=f"x8_{i}")
                for i in range(TT)
            ]
            # W tiles are split into output-column halves: the whole kernel
            # runs as two phases (columns 0:1024, then 1024:2048), so only
            # ~3.75 MB of W must land before phase-0 compute; the other half
            # streams in during phase 0's ~96 us.
            w16t = [
                [
                    wpool.tile([P, OH], fp16, name=f"w16_{k}_{h}", tag=f"w16_{k}_{h}")
                    for h in range(2)
                ]
                for k in range(K16)
            ]
            w8t = [
                [
                    wpool.tile([P, 2, OH], fp8, name=f"w8_{k}_{h}", tag=f"w8_{k}_{h}")
                    for h in range(2)
                ]
                for k in range(KP8)
            ]

            def dma_x(i):
                nc.sync.dma_start(out=x16t[i][:], in_=x16[:, i])
                nc.sync.dma_start(out=x8t[i][:], in_=x8[:, i])

            # All bulk input DMAs stay on one queue (SP) in exact consumption
            # order — a second queue racing ahead starves the ramp-critical
            # W stream. Only tt0's x tiles ride the ScalarE queue, so the
            # first matmul's two dependencies (x0, w16_0) transfer in
            # parallel instead of back-to-back.
            def dma_w16(km, h):
                nc.sync.dma_start(
                    out=w16t[km][h][:], in_=w16[:, km, h * OH : (h + 1) * OH]
                )

            def dma_w8(kp, h):
                nc.sync.dma_start(
                    out=w8t[kp][h][:], in_=w8[:, kp, :, h * OH : (h + 1) * OH]
                )

            # Trigger order tuned so each phase-0 unit's data lands just
            # ahead of its compute (units consume ~620 MB/s vs the ~410 GB/s
            # DMA ceiling, so the early x tiles interleave into the W stream).
            # x16_0 (the first matmul's 0.875 MB long pole) rides the ScalarE
            # queue alone, in parallel with the W stream on SP; x8_0 is not
            # needed until ~10 us after MM0, so it joins the ordered stream.
            nc.scalar.dma_start(out=x16t[0][:], in_=x16[:, 0])
            for km in range(3):
                dma_w16(km, 0)
            nc.sync.dma_start(out=x8t[0][:], in_=x8[:, 0])
            dma_x(1)
            for km in range(3, K16):
                dma_w16(km, 0)
            dma_w8(0, 0)
            dma_w8(1, 0)
            dma_x(2)
            for km in range(6):
                dma_w16(km, 1)
            dma_x(3)
            dma_x(4)
            for km in range(6, K16):
                dma_w16(km, 1)
            dma_w8(0, 1)
            dma_w8(1, 1)
            for i in range(5, TT):
                dma_x(i)

            class Unit:
                """One compute unit: token tile tt, output cols
                [obase*OW, (obase+nseg)*OW), accumulation group per oi seg."""

                def __init__(self, tt, obase, nseg):
                    self.tt, self.obase, self.nseg = tt, obase, nseg
                    self.psums = [
                        ppool.tile(
                            [P, OW], f32, name=f"ps{tt}_{obase}_{oi}", tag="ps"
                        )
                        for oi in range(nseg)
                    ]
                    self.o_sb = opool.tile(
                        [P, nseg * OW], bf16, name=f"o{tt}_{obase}", tag="o"
                    )

                def mm16(self, oi_range, km):
                    for oi in oi_range:
                        o0 = (self.obase + oi) * OW
                        nc.tensor.matmul(
                            out=self.psums[oi][:],
                            lhsT=x16t[self.tt][:, km, :],
                            rhs=w16t[km][(self.obase + oi) // 2][
                                :, o0 % OH : o0 % OH + OW
                            ],
                            start=(km == 0),
                            stop=False,
                        )

                def mm8(self, oi_range, kp):
                    for oi in oi_range:
                        o0 = (self.obase + oi) * OW
                        nc.tensor.matmul(
                            out=self.psums[oi][:],
                            lhsT=x8t[self.tt][:, kp, :, :],
                            rhs=w8t[kp][(self.obase + oi) // 2][
                                :, :, o0 % OH : o0 % OH + OW
                            ],
                            start=False,
                            stop=(kp == KP8 - 1),
                            perf_mode=DR,
                        )

                def fp16_part(self):
                    for km in range(K16):
                        self.mm16(range(self.nseg), km)

                def dr_part(self):
                    for kp in range(KP8):
                        self.mm8(range(self.nseg), kp)

                def copy_out(self, oi):
                    # Dequant copies split across DVE and ScalarE (both can
                    # read PSUM, different banks) so a unit's 4 copies clear
                    # in ~2 copy-times, not 4 — the next-but-one unit's
                    # start=True matmuls wait on these.
                    dst = self.o_sb[:, oi * OW : (oi + 1) * OW]
                    if oi % 2 == 0:
                        nc.vector.tensor_scalar_mul(dst, self.psums[oi][:], DEQ)
                    else:
                        nc.scalar.activation(
                            dst,
                            self.psums[oi][:],
                            mybir.ActivationFunctionType.Copy,
                            scale=DEQ,
                        )

                def finish(self):
                    for oi in range(self.nseg):
                        self.copy_out(oi)
                    nc.scalar.dma_start(
                        out=out[
                            self.tt,
                            :,
                            self.obase * OW : (self.obase + self.nseg) * OW,
                        ],
                        in_=self.o_sb[:],
                    )

            def run_units(units):
                """Pair units so their DR sections run back-to-back: the
                fp16->DR weight-buffer transition costs ~190 ns of PE time
                (a DoubleRow LDWEIGHTS fills both weight slots so it cannot
                prefetch behind fp16 matmuls); pairing halves that count."""
                for i in range(0, len(units) - 1, 2):
                    a, b = units[i], units[i + 1]
                    a.fp16_part()
                    b.fp16_part()
                    a.dr_part()
                    b.dr_part()
                    a.finish()
                    b.finish()
                if len(units) % 2:
                    u = units[-1]
                    u.fp16_part()
                    u.dr_part()
                    u.finish()

            def tail_unit(tt):
                # Tail shape: per-oi groups so copies/DMAs overlap the
                # remaining matmuls; the very last copy splits across both
                # engines (half-width each) to shorten the critical tail.
                u = Unit(tt, 0, ON)
                for oi in range(ON):
                    for km in range(K16):
                        u.mm16([oi], km)
                    for kp in range(KP8):
                        u.mm8([oi], kp)
                    if oi == ON - 1:
                        # Final slice: half-width copies on both engines,
                        # each with its own DMA on its own queue — a single
                        # DMA's pre-wait on the other engine's copy would
                        # serialize the copies (observed on HW), and two
                        # 64 KB transfers drain sooner than one 128 KB.
                        hw = OW // 2
                        nc.vector.tensor_scalar_mul(
                            u.o_sb[:, oi * OW : oi * OW + hw],
                            u.psums[oi][:, :hw],
                            DEQ,
                        )
                        nc.sync.dma_start(
                            out=out[tt, :, oi * OW : oi * OW + hw],
                            in_=u.o_sb[:, oi * OW : oi * OW + hw],
                        )
                        nc.scalar.activation(
                            u.o_sb[:, oi * OW + hw : (oi + 1) * OW],
                            u.psums[oi][:, hw:],
                            mybir.ActivationFunctionType.Copy,
                            scale=DEQ,
                        )
                        nc.scalar.dma_start(
                            out=out[tt, :, oi * OW + hw : (oi + 1) * OW],
                            in_=u.o_sb[:, oi * OW + hw : (oi + 1) * OW],
                        )
                    else:
                        u.copy_out(oi)
                        nc.scalar.dma_start(
                            out=out[tt, :, oi * OW : (oi + 1) * OW],
                            in_=u.o_sb[:, oi * OW : (oi + 1) * OW],
                        )

            # Ramp phase: half-width units over tt0..3 while W streams in,
            # paired as well (also defers the w8 dependency deeper into the
            # ramp); then paired full-width units.
            RAMP_TT = 4
            run_units([Unit(tt, 0, 2) for tt in range(RAMP_TT)])
            run_units([Unit(tt, 2, 2) for tt in range(RAMP_TT)])
            run_units([Unit(tt, 0, ON) for tt in range(RAMP_TT, TT - 1)])
            tail_unit(TT - 1)

    nc.compile()
    _nc_cache["nc"] = nc
    return nc


def _shard_inputs(hidden_states, weight):
    """Host-side quantize + reshuffle into the kernel's DRAM layouts."""
    import ml_dtypes

    fp8 = ml_dtypes.float8_e4m3  # IEEE e4m3, max 240 == TRN FP8_EXP4
    x = np.asarray(hidden_states, dtype=np.float32)
    w = np.asarray(weight, dtype=np.float32)
    k16 = K16 * P  # 1536
    in_maps = []
    for g in range(G):
        xg = x[g * TPG : (g + 1) * TPG]  # [2048, 2048]
        wg = w[g]  # [out, in]
        # fp16 section, k < 1536: [tt, t, km, p] -> [p, tt, km, t]
        x16 = np.ascontiguousarray(
            (xg[:, :k16] * SX)
            .reshape(TT, P, K16, P)
            .transpose(3, 0, 2, 1)
            .astype(np.float16)
        )
        w16 = np.ascontiguousarray(
            (wg[:, :k16] * SW)
            .reshape(OUT, K16, P)
            .transpose(2, 1, 0)
            .astype(np.float16)
        )
        # fp8 section, k >= 1536: [tt, t, kp, i, p] -> [p, tt, kp, i, t]
        x8 = np.ascontiguousarray(
            np.clip(xg[:, k16:] * SX, -240.0, 240.0)
            .reshape(TT, P, KP8, 2, P)
            .transpose(4, 0, 2, 3, 1)
            .astype(fp8)
        )
        w8 = np.ascontiguousarray(
            np.clip(wg[:, k16:] * SW, -240.0, 240.0)
            .reshape(OUT, KP8, 2, P)
            .transpose(3, 1, 2, 0)
            .astype(fp8)
        )
        in_maps.append({"x16": x16, "w16": w16, "x8": x8, "w8": w8})
    return in_maps


def _run(hidden_states, weight, trace=False, tmpdir=None):
    from concourse.bass_utils import run_bass_kernel_spmd

    nc = _build_nc()
    in_maps = _shard_inputs(hidden_states, weight)
    res = run_bass_kernel_spmd(
        nc, in_maps, core_ids=list(range(G)), trace=trace, tmpdir=tmpdir
    )
    outs = [
        np.asarray(res.results[g]["out"]).astype(np.float32).reshape(TPG, OUT)
        for g in range(G)
    ]
    full = np.concatenate(outs, axis=0)
    return full, res


def kernel(hidden_states, weight, tokens_per_expert=None, **_ignored):
    out, _ = _run(hidden_states, weight, trace=False)
    return out


# revision 26
# speedup vs baseline: 1.0208x; 1.0050x over previous
"""Grouped linear (MoE expert GEMM) on 8 NeuronCores, expert-parallel.

Problem: hidden_states [16384, 2048] f32, weight [8, 2048, 2048] f32,
tokens_per_expert [8] = 2048 each (balanced). Output [16384, 2048] f32 with
out[g*2048+t, o] = sum_i x[g*2048+t, i] * weight[g, o, i].

Sharding: expert-parallel -- core g gets expert g's weight [2048, 2048] and its
2048 routed tokens; each core runs one 2048x2048x2048 GEMM. No collectives.

Per-core kernel, mixed precision to beat the 1-col/cycle PE floor:
- k 0..1535 (12 chunks of 128) in fp16: 1 col/cycle, 216 ns per 512-wide MM.
- k 1536..2047 (2 pairs of 256) in fp8-e4m3 with perf_mode=DoubleRow:
  2 MACs/cell/cycle, ~109 ns per 512-wide MM covering 256 k.
Both sections accumulate into one PSUM group: all operands carry a shared
power-of-2 scale (x*32, w*8192 -> PSUM holds 2^18 * out), removed by a
tensor_scalar_mul(2^-18) in the PSUM->SBUF copy. Host-simulated rel err on
the real data: 1.63e-2 (gate 2e-2); fp8 quantization dominates.

The loop is ordered (tt, km, oi) so each stationary tile is reused for 4
consecutive matmuls (LDWEIGHTS amortized/hidden). DMA trigger order is the
ramp-critical path: x(tt=0) then all W tiles, then x1..x15. Output is bf16,
batched one DMA per token tile; host upcasts.
"""

import numpy as np

G = 8
TPG = 2048  # tokens per expert (= per core)
IN = 2048
OUT = 2048
P = 128
TT = TPG // P  # 16 token tiles of 128
ON = 4  # number of output-column chunks
OW = OUT // ON  # 512
K16 = 12  # fp16 contraction chunks of 128 (k 0..1535)
KP8 = 2  # fp8 DoubleRow pairs of 256 (k 1536..2047)
SX = 32.0  # power-of-2 scale on x (both sections)
SW = 8192.0  # power-of-2 scale on w (both sections)
DEQ = 1.0 / (SX * SW)  # 2^-18

_nc_cache = {}


def _build_nc():
    import concourse.bacc as bacc
    import concourse.mybir as mybir
    import concourse.tile as tile

    if "nc" in _nc_cache:
        return _nc_cache["nc"]

    f32 = mybir.dt.float32
    bf16 = mybir.dt.bfloat16
    fp16 = mybir.dt.float16
    fp8 = mybir.dt.float8e4
    DR = mybir.MatmulPerfMode.DoubleRow

    nc = bacc.Bacc(None, target_bir_lowering=False)

    # x16[p, tt, km, t] = SX * x[tt*128+t, km*128+p]          (k on partitions)
    x16 = nc.dram_tensor("x16", [P, TT, K16, P], fp16, kind="ExternalInput")
    # w16[p, km, o] = SW * w[o, km*128+p]
    w16 = nc.dram_tensor("w16", [P, K16, OUT], fp16, kind="ExternalInput")
    # x8[p, tt, kp, i, t] = q(SX * x[tt*128+t, 1536 + kp*256 + i*128 + p])
    x8 = nc.dram_tensor("x8", [P, TT, KP8, 2, P], fp8, kind="ExternalInput")
    # w8[p, kp, i, o] = q(SW * w[o, 1536 + kp*256 + i*128 + p])
    w8 = nc.dram_tensor("w8", [P, KP8, 2, OUT], fp8, kind="ExternalInput")
    # out[tt, p, o] = C[tt*128+p, o] (bf16; host upcasts)
    out = nc.dram_tensor("out", [TT, P, OUT], bf16, kind="ExternalOutput")

    with tile.TileContext(nc) as tc:
        with (
            tc.tile_pool(name="xpool", bufs=1) as xpool,
            tc.tile_pool(name="wpool", bufs=1) as wpool,
            tc.tile_pool(name="opool", bufs=2) as opool,
            tc.tile_pool(name="ppool", bufs=8, space="PSUM") as ppool,
        ):
            OH = 2 * OW  # output half-width: 1024
            x16t = [
                xpool.tile([P, K16, P], fp16, name=f"x16_{i}", tag=f"x16_{i}")
                for i in range(TT)
            ]
            x8t = [
                xpool.tile([P, KP8, 2, P], fp8, name=f"x8_{i}", tag=f"x8_{i}")
                for i in range(TT)
            ]
            # W tiles are split into output-column halves: the whole kernel
            # runs as two phases (columns 0:1024, then 1024:2048), so only
            # ~3.75 MB of W must land before phase-0 compute; the other half
            # streams in during phase 0's ~96 us.
            w16t = [
                [
                    wpool.tile([P, OH], fp16, name=f"w16_{k}_{h}", tag=f"w16_{k}_{h}")
                    for h in range(2)
                ]
                for k in range(K16)
            ]
            w8t = [
                [
                    wpool.tile([P, 2, OH], fp8, name=f"w8_{k}_{h}", tag=f"w8_{k}_{h}")
                    for h in range(2)
                ]
                for k in range(KP8)
            ]

            def dma_x(i):
                nc.sync.dma_start(out=x16t[i][:], in_=x16[:, i])
                nc.sync.dma_start(out=x8t[i][:], in_=x8[:, i])

            # All bulk input DMAs stay on one queue (SP) in exact consumption
            # order — a second queue racing ahead starves the ramp-critical
            # W stream. Only tt0's x tiles ride the ScalarE queue, so the
            # first matmul's two dependencies (x0, w16_0) transfer in
            # parallel instead of back-to-back.
            def dma_w16(km, h):
                nc.sync.dma_start(
                    out=w16t[km][h][:], in_=w16[:, km, h * OH : (h + 1) * OH]
                )

            def dma_w8(kp, h):
                nc.sync.dma_start(
                    out=w8t[kp][h][:], in_=w8[:, kp, :, h * OH : (h + 1) * OH]
                )

            # Trigger order tuned so each phase-0 unit's data lands just
            # ahead of its compute (units consume ~620 MB/s vs the ~410 GB/s
            # DMA ceiling, so the early x tiles interleave into the W stream).
            # x16_0 (the first matmul's 0.875 MB long pole) rides the ScalarE
            # queue alone, in parallel with the W stream on SP; x8_0 is not
            # needed until ~10 us after MM0, so it joins the ordered stream.
            nc.scalar.dma_start(out=x16t[0][:], in_=x16[:, 0])
            for km in range(3):
                dma_w16(km, 0)
            nc.sync.dma_start(out=x8t[0][:], in_=x8[:, 0])
            dma_x(1)
            for km in range(3, K16):
                dma_w16(km, 0)
            dma_w8(0, 0)
            dma_w8(1, 0)
            dma_x(2)
            for km in range(6):
                dma_w16(km, 1)
            dma_x(3)
            dma_x(4)
            for km in range(6, K16):
                dma_w16(km, 1)
            dma_w8(0, 1)
            dma_w8(1, 1)
            for i in range(5, TT):
                dma_x(i)

            class Unit:
                """One compute unit: token tile tt, output cols
                [obase*OW, (obase+nseg)*OW), accumulation group per oi seg."""

                def __init__(self, tt, obase, nseg):
                    self.tt, self.obase, self.nseg = tt, obase, nseg
                    self.psums = [
                        ppool.tile(
                            [P, OW], f32, name=f"ps{tt}_{obase}_{oi}", tag="ps"
                        )
                        for oi in range(nseg)
                    ]
                    self.o_sb = opool.tile(
                        [P, nseg * OW], bf16, name=f"o{tt}_{obase}", tag="o"
                    )

                def mm16(self, oi_range, km):
                    for oi in oi_range:
                        o0 = (self.obase + oi) * OW
                        nc.tensor.matmul(
                            out=self.psums[oi][:],
                            lhsT=x16t[self.tt][:, km, :],
                            rhs=w16t[km][(self.obase + oi) // 2][
                                :, o0 % OH : o0 % OH + OW
                            ],
                            start=(km == 0),
                            stop=False,
                        )

                def mm8(self, oi_range, kp):
                    for oi in oi_range:
                        o0 = (self.obase + oi) * OW
                        nc.tensor.matmul(
                            out=self.psums[oi][:],
                            lhsT=x8t[self.tt][:, kp, :, :],
                            rhs=w8t[kp][(self.obase + oi) // 2][
                                :, :, o0 % OH : o0 % OH + OW
                            ],
                            start=False,
                            stop=(kp == KP8 - 1),
                            perf_mode=DR,
                        )

                def fp16_part(self):
                    for km in range(K16):
                        self.mm16(range(self.nseg), km)

                def dr_part(self):
                    for kp in range(KP8):
                        self.mm8(range(self.nseg), kp)

                def copy_out(self, oi):
                    # Dequant copies split across DVE and ScalarE (both can
                    # read PSUM, different banks) so a unit's 4 copies clear
                    # in ~2 copy-times, not 4 — the next-but-one unit's
                    # start=True matmuls wait on these.
                    dst = self.o_sb[:, oi * OW : (oi + 1) * OW]
                    if oi % 2 == 0:
                        nc.vector.tensor_scalar_mul(dst, self.psums[oi][:], DEQ)
                    else:
                        nc.scalar.activation(
                            dst,
                            self.psums[oi][:],
                            mybir.ActivationFunctionType.Copy,
                            scale=DEQ,
                        )

                def finish(self):
                    for oi in range(self.nseg):
                        self.copy_out(oi)
                    nc.scalar.dma_start(
                        out=out[
                            self.tt,
                            :,
                            self.obase * OW : (self.obase + self.nseg) * OW,
                        ],
                        in_=self.o_sb[:],
                    )

            def run_units(units):
                """Pair units so their DR sections run back-to-back: the
                fp16->DR weight-buffer transition costs ~190 ns of PE time
                (a DoubleRow LDWEIGHTS fills both weight slots so it cannot
                prefetch behind fp16 matmuls); pairing halves that count."""
                for i in range(0, len(units) - 1, 2):
                    a, b = units[i], units[i + 1]
                    a.fp16_part()
                    b.fp16_part()
                    a.dr_part()
                    b.dr_part()
                    a.finish()
                    b.finish()
                if len(units) % 2:
                    u = units[-1]
                    u.fp16_part()
                    u.dr_part()
                    u.finish()

            def tail_unit(tt):
                # Tail shape: per-oi groups so copies/DMAs overlap the
                # remaining matmuls; the very last copy splits across both
                # engines (half-width each) to shorten the critical tail.
                u = Unit(tt, 0, ON)
                for oi in range(ON):
                    for km in range(K16):
                        u.mm16([oi], km)
                    for kp in range(KP8):
                        u.mm8([oi], kp)
                    if oi == ON - 1:
                        # Final slice: half-width copies on both engines into
                        # SEPARATE tiles (tile-granular dep tracking would
                        # otherwise serialize them), each with its own DMA on
                        # its own queue; two 64 KB transfers drain sooner
                        # than one 128 KB.
                        hw = OW // 2
                        o_l = opool.tile([P, hw], bf16, name="o_l", tag="ol")
                        o_r = opool.tile([P, hw], bf16, name="o_r", tag="or")
                        nc.vector.tensor_scalar_mul(
                            o_l[:], u.psums[oi][:, :hw], DEQ
                        )
                        nc.sync.dma_start(
                            out=out[tt, :, oi * OW : oi * OW + hw], in_=o_l[:]
                        )
                        nc.scalar.activation(
                            o_r[:],
                            u.psums[oi][:, hw:],
                            mybir.ActivationFunctionType.Copy,
                            scale=DEQ,
                        )
                        nc.scalar.dma_start(
                            out=out[tt, :, oi * OW + hw : (oi + 1) * OW],
                            in_=o_r[:],
                        )
                    else:
                        u.copy_out(oi)
                        nc.scalar.dma_start(
                            out=out[tt, :, oi * OW : (oi + 1) * OW],
                            in_=u.o_sb[:, oi * OW : (oi + 1) * OW],
                        )

            # Ramp phase: half-width units over tt0..3 while W streams in,
            # paired as well (also defers the w8 dependency deeper into the
            # ramp); then paired full-width units.
            RAMP_TT = 4
            run_units([Unit(tt, 0, 2) for tt in range(RAMP_TT)])
            run_units([Unit(tt, 2, 2) for tt in range(RAMP_TT)])
            run_units([Unit(tt, 0, ON) for tt in range(RAMP_TT, TT - 1)])
            tail_unit(TT - 1)

    nc.compile()
    _nc_cache["nc"] = nc
    return nc


def _shard_inputs(hidden_states, weight):
    """Host-side quantize + reshuffle into the kernel's DRAM layouts."""
    import ml_dtypes

    fp8 = ml_dtypes.float8_e4m3  # IEEE e4m3, max 240 == TRN FP8_EXP4
    x = np.asarray(hidden_states, dtype=np.float32)
    w = np.asarray(weight, dtype=np.float32)
    k16 = K16 * P  # 1536
    in_maps = []
    for g in range(G):
        xg = x[g * TPG : (g + 1) * TPG]  # [2048, 2048]
        wg = w[g]  # [out, in]
        # fp16 section, k < 1536: [tt, t, km, p] -> [p, tt, km, t]
        x16 = np.ascontiguousarray(
            (xg[:, :k16] * SX)
            .reshape(TT, P, K16, P)
            .transpose(3, 0, 2, 1)
            .astype(np.float16)
        )
        w16 = np.ascontiguousarray(
            (wg[:, :k16] * SW)
            .reshape(OUT, K16, P)
            .transpose(2, 1, 0)
            .astype(np.float16)
        )
        # fp8 section, k >= 1536: [tt, t, kp, i, p] -> [p, tt, kp, i, t]
        x8 = np.ascontiguousarray(
            np.clip(xg[:, k16:] * SX, -240.0, 240.0)
            .reshape(TT, P, KP8, 2, P)
            .transpose(4, 0, 2, 3, 1)
            .astype(fp8)
        )
        w8 = np.ascontiguousarray(
            np.clip(wg[:, k16:] * SW, -240.0, 240.0)
            .reshape(OUT, KP8, 2, P)
            .transpose(3, 1, 2, 0)
            .astype(fp8)
        )
        in_maps.append({"x16": x16, "w16": w16, "x8": x8, "w8": w8})
    return in_maps


def _run(hidden_states, weight, trace=False, tmpdir=None):
    from concourse.bass_utils import run_bass_kernel_spmd

    nc = _build_nc()
    in_maps = _shard_inputs(hidden_states, weight)
    res = run_bass_kernel_spmd(
        nc, in_maps, core_ids=list(range(G)), trace=trace, tmpdir=tmpdir
    )
    outs = [
        np.asarray(res.results[g]["out"]).astype(np.float32).reshape(TPG, OUT)
        for g in range(G)
    ]
    full = np.concatenate(outs, axis=0)
    return full, res


def kernel(hidden_states, weight, tokens_per_expert=None, **_ignored):
    out, _ = _run(hidden_states, weight, trace=False)
    return out
